# revision 7
# baseline (speedup 1.0000x reference)
"""Multihead attention (B=2, S=2048, D=1024, H=16) on 8 TRN2 NeuronCores.

Sharding: core c -> batch b = c//4, head-group g = c%4 (4 heads, 256 features).
Each core computes q/k/v projections for its 256 features, attention for its
4 heads, and a row-parallel partial of the output projection. Host sums the
4 partials per batch (row-parallel TP unshard) and transposes back.

On-device layout (per core):
  xqT/xkT/xvT : [1024, 2048]  input.T (bf16)
  qT, kT      : [256, 2048]   projected, feature-major (scores need K/Q as lhsT/rhs
                              with head-dim on partitions)
  va          : [2048, 4*65]  v in [s, f] layout, per head 64 v-cols + ones col
                              (ones col makes the AV matmul emit the softmax
                              denominator as row 64 of its PSUM output)
  scoresT     : [128 j, 2048 i] PSUM per j-tile; exp(scale*x + mask_j) fused in
                              one ScalarE activation (mask is a per-partition bias)
  out partial : [1024, 2048]  (= partial_out.T, f32)
"""

import math

import numpy as np

B, S, D, H = 2, 2048, 1024, 16
NCORES = 8
GH = 4                  # heads per core
HD = D // H             # 64
F = GH * HD             # 256 local features
SCALE = 1.0 / math.sqrt(HD)
NEG = np.float32(-9e9)

KT = D // 128           # 8 contraction tiles (projections)
FT = F // 128           # 2 local-feature tiles
ST = S // 128           # 16 sequence tiles
DT = D // 128           # 8 output-feature tiles

TRACE = False           # set by test harness; requires antenv.axon_hooks wired
LAST_EXEC_NS = None
LAST_RESULTS = None

_STATE = {}


def _build():
    import concourse.bacc as bacc
    import concourse.mybir as mybir
    from concourse import masks
    from concourse.tile import TileContext

    f32 = mybir.dt.float32
    bf16 = mybir.dt.bfloat16
    Exp = mybir.ActivationFunctionType.Exp

    nc = bacc.Bacc("TRN2", target_bir_lowering=False, debug=False,
                   num_devices=NCORES)

    xq_d = nc.declare_dram_parameter("xqT", [D, S], bf16, isOutput=False)
    xk_d = nc.declare_dram_parameter("xkT", [D, S], bf16, isOutput=False)
    xv_d = nc.declare_dram_parameter("xvT", [D, S], bf16, isOutput=False)
    wq_d = nc.declare_dram_parameter("wqT", [D, F], bf16, isOutput=False)
    wk_d = nc.declare_dram_parameter("wkT", [D, F], bf16, isOutput=False)
    wv_d = nc.declare_dram_parameter("wvT", [D, F], bf16, isOutput=False)
    wo_d = nc.declare_dram_parameter("woT", [F, D], bf16, isOutput=False)
    bq_d = nc.declare_dram_parameter("bq", [F], f32, isOutput=False)
    bk_d = nc.declare_dram_parameter("bk", [F], f32, isOutput=False)
    bv_d = nc.declare_dram_parameter("bv", [F], bf16, isOutput=False)
    bo_d = nc.declare_dram_parameter("bo", [D], f32, isOutput=False)
    mk_d = nc.declare_dram_parameter("maskf", [S], f32, isOutput=False)
    out_d = nc.declare_dram_parameter("outT", [D, S], f32, isOutput=True)

    with TileContext(nc) as tc:
        with tc.tile_pool(name="persist", bufs=1) as pp, \
             tc.tile_pool(name="xin", bufs=10) as xp, \
             tc.tile_pool(name="expp", bufs=18) as ep, \
             tc.tile_pool(name="ostage", bufs=3) as osp, \
             tc.tile_pool(name="small", bufs=6) as sp:

            def ptile(shape, dtype, name):
                return pp.tile(shape, dtype, name=name, tag=name)

            # ---- persistent SBUF tensors ----
            wq_sb = [ptile([128, F], bf16, f"wq{k}") for k in range(KT)]
            wk_sb = [ptile([128, F], bf16, f"wk{k}") for k in range(KT)]
            wv_sb = [ptile([128, F], bf16, f"wv{k}") for k in range(KT)]
            wo_sb = [ptile([128, D], bf16, f"wo{t}") for t in range(FT)]
            bq_sb = [ptile([128, 1], f32, f"bq{t}") for t in range(FT)]
            bk_sb = [ptile([128, 1], f32, f"bk{t}") for t in range(FT)]
            bo_sb = [ptile([128, 1], f32, f"bo{t}") for t in range(DT)]
            mk_sb = [ptile([128, 1], f32, f"mk{j}") for j in range(ST)]
            bv_sb = ptile([1, F], bf16, "bvrow")
            ones_sb = ptile([1, 128], bf16, "onesrow")
            ident = ptile([128, 128], bf16, "ident")
            qT_sb = [ptile([128, S], bf16, f"qT{t}") for t in range(FT)]
            kT_sb = [ptile([128, S], bf16, f"kT{t}") for t in range(FT)]
            va_sb = [ptile([128, GH * (HD + 1)], bf16, f"va{j}") for j in range(ST)]
            os_sb = [ptile([128, F], bf16, f"os{i}") for i in range(ST)]
            ot_sb = [ptile([128, S], bf16, f"ot{t}") for t in range(FT)]

            nc.vector.memset(ones_sb[:], 1.0)
            masks.make_identity(nc, ident[:])
            for j in range(ST):
                nc.vector.memset(va_sb[j][:], 1.0)

            # DMAs emitted in consumption order so the first projection can
            # start as soon as its first k-tile lands.
            xq_sb, xk_sb, xv_sb = [], [], []
            for w_sb, w_d, x_sb, x_d in ((wq_sb, wq_d, xq_sb, xq_d),
                                         (wk_sb, wk_d, xk_sb, xk_d),
                                         (wv_sb, wv_d, xv_sb, xv_d)):
                for k in range(KT):
                    nc.sync.dma_start(out=w_sb[k][:],
                                      in_=w_d[k * 128:(k + 1) * 128, :])
                    xt = xp.tile([128, S], bf16, name=f"x{k}", tag="xin")
                    nc.sync.dma_start(out=xt[:], in_=x_d[k * 128:(k + 1) * 128, :])
                    x_sb.append(xt)
            for t in range(FT):
                nc.sync.dma_start(out=wo_sb[t][:], in_=wo_d[t * 128:(t + 1) * 128, :])
                nc.sync.dma_start(out=bq_sb[t][:],
                                  in_=bq_d[t * 128:(t + 1) * 128].unsqueeze(1))
                nc.sync.dma_start(out=bk_sb[t][:],
                                  in_=bk_d[t * 128:(t + 1) * 128].unsqueeze(1))
            for t in range(DT):
                nc.sync.dma_start(out=bo_sb[t][:],
                                  in_=bo_d[t * 128:(t + 1) * 128].unsqueeze(1))
            for j in range(ST):
                nc.sync.dma_start(out=mk_sb[j][:],
                                  in_=mk_d[j * 128:(j + 1) * 128].unsqueeze(1))
            nc.sync.dma_start(out=bv_sb[:], in_=bv_d[:].unsqueeze(0))

            # ---- stage A: projections ----
            with tc.tile_pool(name="psA", bufs=4, space="PSUM") as psA:
                # q/k projections, feature-major output: out[f, s].
                # k-outer: all 4 (f-tile, s-half) accumulators live in PSUM so
                # each input k-tile is consumed the moment its DMA lands.
                for w_sb, x_sb, b_sb, y_sb in ((wq_sb, xq_sb, bq_sb, qT_sb),
                                               (wk_sb, xk_sb, bk_sb, kT_sb)):
                    pss = [psA.tile([128, 1024], mybir.dt.float32,
                                    name=f"psqk{t}{sh}", tag="psqk")
                           for t in range(FT) for sh in range(2)]
                    for k in range(KT):
                        for t in range(FT):
                            for sh in range(2):
                                ps = pss[t * 2 + sh]
                                s0 = sh * 1024
                                for n in range(2):
                                    nc.tensor.matmul(
                                        ps[:, n * 512:(n + 1) * 512],
                                        lhsT=w_sb[k][:, t * 128:(t + 1) * 128],
                                        rhs=x_sb[k][:, s0 + n * 512:s0 + (n + 1) * 512],
                                        start=(k == 0), stop=(k == KT - 1))
                    for t in range(FT):
                        for sh in range(2):
                            nc.vector.tensor_scalar_add(
                                y_sb[t][:, sh * 1024:(sh + 1) * 1024],
                                pss[t * 2 + sh][:], b_sb[t][:])
                # v projection, sequence-major output: out[s, f] (+bias via ones row)
                for st in range(ST):
                    pv = psA.tile([128, F], mybir.dt.float32, name="psv", tag="psqk")
                    for k in range(KT):
                        nc.tensor.matmul(
                            pv[:], lhsT=xv_sb[k][:, st * 128:(st + 1) * 128],
                            rhs=wv_sb[k][:], start=(k == 0), stop=False)
                    nc.tensor.matmul(pv[:], lhsT=ones_sb[:], rhs=bv_sb[:],
                                     start=False, stop=True)
                    for h in range(GH):
                        nc.vector.tensor_copy(
                            va_sb[st][:, h * (HD + 1):h * (HD + 1) + HD],
                            pv[:, h * HD:(h + 1) * HD])

            # ---- stage B: attention per head; C/D folded into last head ----
            with tc.tile_pool(name="psB", bufs=2, space="PSUM") as psB:

                def transpose_o(it):
                    for t in range(FT):
                        pt = psB.tile([128, 128], bf16, name="pst", tag="pssc")
                        nc.tensor.transpose(
                            pt[:], os_sb[it][:, t * 128:(t + 1) * 128], ident[:])
                        nc.vector.tensor_copy(
                            ot_sb[t][:, it * 128:(it + 1) * 128], pt[:])

                def out_proj(ih):
                    i0 = ih * 1024
                    for do in range(DT):
                        pso = psB.tile([128, 1024], mybir.dt.float32,
                                       name="pso", tag="pssc")
                        for n in range(2):
                            for t in range(FT):
                                nc.tensor.matmul(
                                    pso[:, n * 512:(n + 1) * 512],
                                    lhsT=wo_sb[t][:, do * 128:(do + 1) * 128],
                                    rhs=ot_sb[t][:, i0 + n * 512:i0 + (n + 1) * 512],
                                    start=(t == 0), stop=(t == FT - 1))
                        stg = osp.tile([128, 1024], mybir.dt.float32,
                                       name="stg", tag="stg")
                        # alternate PSUM-evacuation between DVE and ACT (ACT is
                        # idle in the tail) so copies don't serialize on DVE
                        if do % 2 == 0:
                            nc.vector.tensor_scalar_add(stg[:], pso[:], bo_sb[do][:])
                        else:
                            nc.scalar.add(stg[:], pso[:], bo_sb[do][:])
                        nc.sync.dma_start(
                            out=out_d[do * 128:(do + 1) * 128, i0:i0 + 1024],
                            in_=stg[:])

                def emit_scores(h, j):
                    ht = h // 2
                    off = (h % 2) * HD
                    ps = psB.tile([128, S], mybir.dt.float32,
                                  name="pssc", tag="pssc")
                    for n in range(4):
                        nc.tensor.matmul(
                            ps[:, n * 512:(n + 1) * 512],
                            lhsT=kT_sb[ht][off:off + HD, j * 128:(j + 1) * 128],
                            rhs=qT_sb[ht][off:off + HD, n * 512:(n + 1) * 512],
                            start=True, stop=True)
                    e = ep.tile([128, S], bf16, name="expT", tag="expT")
                    nc.scalar.activation(e[:], ps[:], Exp,
                                         bias=mk_sb[j][:], scale=SCALE)
                    return e

                def emit_av(h, ets):
                    for it in range(ST):
                        po = psB.tile([128, HD + 1], mybir.dt.float32,
                                      name="psav", tag="pssc")
                        for j in range(ST):
                            nc.tensor.matmul(
                                po[:],
                                lhsT=ets[j][:, it * 128:(it + 1) * 128],
                                rhs=va_sb[j][:, h * (HD + 1):(h + 1) * (HD + 1)],
                                start=(j == 0), stop=(j == ST - 1))
                        rec = sp.tile([128, 1], mybir.dt.float32,
                                      name="rec", tag="rec")
                        nc.vector.reciprocal(rec[:], po[:, HD:HD + 1])
                        nc.vector.tensor_scalar_mul(
                            os_sb[it][:, h * HD:(h + 1) * HD],
                            po[:, 0:HD], rec[:])
                        if h == GH - 1:
                            transpose_o(it)
                            if it == ST // 2 - 1:
                                out_proj(0)
                            elif it == ST - 1:
                                out_proj(1)

                # software pipeline: the first two score tiles (and their exps)
                # of head h are emitted before av(h-1), so ScalarE has work
                # while the PE drains the previous head's AV matmuls.
                prev = None
                for h in range(GH):
                    ets = [emit_scores(h, j) for j in range(2)]
                    if prev is not None:
                        emit_av(h - 1, prev)
                    ets += [emit_scores(h, j) for j in range(2, ST)]
                    prev = ets
                emit_av(GH - 1, prev)

    nc.compile()
    return nc


def kernel(query, key, value, src_mask, Wq, bq, Wk, bk, Wv, bv, Wo, bo, nhead):
    global LAST_EXEC_NS, LAST_RESULTS
    import ml_dtypes
    from concourse.bass_utils import run_bass_kernel_spmd

    assert int(nhead) == H
    bf16 = ml_dtypes.bfloat16
    query = np.asarray(query, dtype=np.float32)
    key = np.asarray(key, dtype=np.float32)
    value = np.asarray(value, dtype=np.float32)
    src_mask = np.asarray(src_mask)
    Wq, bq = np.asarray(Wq, np.float32), np.asarray(bq, np.float32)
    Wk, bk = np.asarray(Wk, np.float32), np.asarray(bk, np.float32)
    Wv, bv = np.asarray(Wv, np.float32), np.asarray(bv, np.float32)
    Wo, bo = np.asarray(Wo, np.float32), np.asarray(bo, np.float32)

    if "nc" not in _STATE:
        _STATE["nc"] = _build()
    nc = _STATE["nc"]

    xqT = [np.ascontiguousarray(query[b].T).astype(bf16) for b in range(B)]
    xkT = [np.ascontiguousarray(key[b].T).astype(bf16) for b in range(B)]
    xvT = [np.ascontiguousarray(value[b].T).astype(bf16) for b in range(B)]
    maskf = [np.where(src_mask[b], NEG, np.float32(0)).astype(np.float32)
             for b in range(B)]

    wqT, wkT, wvT, woT, bqs, bks, bvs = [], [], [], [], [], [], []
    for g in range(NCORES // B):
        gs, ge = g * F, (g + 1) * F
        wqT.append(np.ascontiguousarray(Wq[gs:ge, :].T).astype(bf16))
        wkT.append(np.ascontiguousarray(Wk[gs:ge, :].T).astype(bf16))
        wvT.append(np.ascontiguousarray(Wv[gs:ge, :].T).astype(bf16))
        woT.append(np.ascontiguousarray(Wo[:, gs:ge].T).astype(bf16))
        bqs.append(np.ascontiguousarray(bq[gs:ge]))
        bks.append(np.ascontiguousarray(bk[gs:ge]))
        bvs.append(bv[gs:ge].astype(bf16))
    bo_zero = np.zeros_like(bo)

    in_maps = []
    for c in range(NCORES):
        b, g = c // (NCORES // B), c % (NCORES // B)
        in_maps.append({
            "xqT": xqT[b], "xkT": xkT[b], "xvT": xvT[b],
            "wqT": wqT[g], "wkT": wkT[g], "wvT": wvT[g], "woT": woT[g],
            "bq": bqs[g], "bk": bks[g], "bv": bvs[g],
            "bo": bo if g == 0 else bo_zero,
            "maskf": maskf[b],
        })

    kwargs = {}
    if TRACE:
        kwargs = dict(trace=True)
    res = run_bass_kernel_spmd(nc, in_maps, core_ids=list(range(NCORES)),
                               **kwargs)
    LAST_EXEC_NS = res.exec_time_ns
    LAST_RESULTS = res

    out = np.empty((B, S, D), dtype=np.float32)
    for b in range(B):
        acc = res.results[b * (NCORES // B)]["outT"].astype(np.float32)
        for g in range(1, NCORES // B):
            acc = acc + res.results[b * (NCORES // B) + g]["outT"]
        out[b] = acc.T
    return out
